# revision 1
# baseline (speedup 1.0000x reference)
"""DoRA linear layer on 8 TRN2 NeuronCores.

out = (magnitude / ||W + s*B@A||_row) * (x @ (W + s*B@A)^T),  s = alpha/rank = 2.

Identity used: the reference's
    dora_out + base_out = mag_norm_scale * (base_out + s * lora_out)
                        = scale_o * (x @ W_adapted^T)
so the kernel runs ONE big fp32r matmul x @ W_ad^T (with the rank-16 term
added as an extra PSUM-accumulated matmul) and a per-out-column scale.

Sharding: data-parallel on tokens (8192 tokens -> 1024/core); W/A/B/mag
replicated. Host side only reshapes/transposes (layout prep) and rounds
fp32 -> fp32r bit format (the dtype the tensor engine consumes).

Row norms of W_ad are computed on-device from the expansion
  ||W + B2@A||^2_row = rowsum(W*W) + 2*rowsum((W@A^T) * B2) + rowsum((B2@G) * B2)
with B2 = s*B, G = A@A^T.  rowsum(W*W) and W@A^T come from one fused fp16
matmul per W^T tile (gram diag + cross term), everything else is tiny.
"""

import sys

sys.path.insert(0, "/opt/trn_rl_repo")

import numpy as np

import concourse.bass as bass  # noqa: F401  (import keeps bass registered)
from concourse import bacc
import concourse.mybir as mybir
from concourse.tile import TileContext
from concourse.bass_utils import run_bass_kernel_spmd
from concourse.masks import make_identity

FP32 = mybir.dt.float32
F32R = mybir.dt.float32r
FP16 = mybir.dt.float16

NCORES = 8
TOK = 8192          # 4 * 2048 tokens
TPC = TOK // NCORES  # 1024 tokens per core
DIN = 4096
DOUT = 4096
RANK = 16
SCALING = 32.0 / 16

NI = DIN // 128     # 32 contraction blocks
NCOL = 8            # output columns of 512
OC = DOUT // NCOL   # 512
NT = TPC // 128     # 8 token tiles per core
H = 8               # ib-chunk size (W-tile working set)
NH = NI // H        # 4 chunks per column


def _round_f32r(x: np.ndarray) -> np.ndarray:
    """Round-to-nearest-even fp32 -> fp32r bit format (11 explicit mantissa
    bits, low 12 bits zero) — matches the PE's own input rounding."""
    u = np.ascontiguousarray(x, dtype=np.float32).view(np.uint32)
    odd = (u >> np.uint32(12)) & np.uint32(1)
    r = (u + np.uint32(0x7FF) + odd) & np.uint32(0xFFFFF000)
    return r.view(np.float32)


def _build_program(ncol_limit=NCOL, skip_prologue=False):
    nc = bacc.Bacc("TRN2", target_bir_lowering=False, debug=False,
                   num_devices=NCORES)

    xt_d = nc.dram_tensor("xt", [128, NI, TPC], FP32, kind="ExternalInput")
    wt_d = nc.dram_tensor("wt", [NCOL, NI, 128, OC], FP32, kind="ExternalInput")
    wh_d = nc.dram_tensor("wh", [128, 32, DIN], FP16, kind="ExternalInput")
    at_d = nc.dram_tensor("at", [128, NI, RANK], FP32, kind="ExternalInput")
    b2t_d = nc.dram_tensor("b2t", [RANK, DOUT], FP32, kind="ExternalInput")
    mag_d = nc.dram_tensor("mag", [1, DOUT], FP32, kind="ExternalInput")
    out_d = nc.dram_tensor("out", [TPC, DOUT], FP32, kind="ExternalOutput")
    srow_d = nc.dram_tensor("srow_scratch", [NCOL, OC], FP32)
    n1_d = nc.dram_tensor("n1_scratch", [NCOL, OC], FP32)

    with TileContext(nc) as tc:
        with (
            tc.tile_pool(name="const", bufs=1) as const,
            tc.tile_pool(name="xtp", bufs=1) as xtp,
            tc.tile_pool(name="wp", bufs=10) as wp,
            tc.tile_pool(name="outp", bufs=10) as outp,
            tc.tile_pool(name="whp", bufs=3) as whp,
            tc.tile_pool(name="b2tp", bufs=2) as b2tp,
            tc.tile_pool(name="sbcp", bufs=2) as sbcp,
            tc.tile_pool(name="mp", bufs=3, space="PSUM") as mp,
            tc.tile_pool(name="np", bufs=2, space="PSUM") as npp,
        ):
            ident = const.tile([128, 128], FP32)
            make_identity(nc, ident)

            aT = const.tile([128, NI, RANK], F32R)
            nc.sync.dma_start(aT[:], at_d[:].bitcast(F32R))
            ones16 = const.tile([RANK, 1], FP32)
            nc.vector.memset(ones16[:], 1.0)

            # resident x^T  [i_part, i_blk, tok] — four tiles so consumers
            # of early i-blocks need not wait for the whole 16 MiB load
            xTq = []
            for q in range(4):
                xq = xtp.tile([128, 8, TPC], F32R, name=f"xTq{q}")
                nc.sync.dma_start(xq[:], xt_d[:, q * 8:(q + 1) * 8, :].bitcast(F32R))
                xTq.append(xq)

            def xT(ib):
                return xTq[ib // 8][:, ib % 8, :]

            # xa^T = (x @ A^T)^T  [rank, tok]
            xaT = const.tile([RANK, TPC], F32R)
            for q in range(2):
                ps_xa = mp.tile([RANK, 512], FP32, tag="mp", name=f"psxa{q}")
                for ib in range(NI):
                    nc.tensor.matmul(
                        ps_xa[:], aT[:, ib, :], xT(ib)[:, q * 512:(q + 1) * 512],
                        start=(ib == 0), stop=(ib == NI - 1))
                nc.vector.tensor_copy(xaT[:, q * 512:(q + 1) * 512], ps_xa[:])

            # G = A @ A^T  [rank, rank]
            ps_g = mp.tile([RANK, RANK], FP32, tag="mp", name="psg")
            for ib in range(NI):
                nc.tensor.matmul(ps_g[:], aT[:, ib, :], aT[:, ib, :],
                                 start=(ib == 0), stop=(ib == NI - 1))
            g_sb = const.tile([RANK, RANK], F32R)
            nc.vector.tensor_copy(g_sb[:], ps_g[:])

            # n1 = rowsum(W*W) per out row, via ACT Square+accumulate over a
            # fp16 copy of W in natural layout; 4 chunk-partials per subtile
            n1p = const.tile([128, 4], FP32)
            n1col = const.tile([128, 4], FP32)
            n1row = const.tile([4, 128], FP32)
            # row-space [1, OC] norm pieces
            prod2 = const.tile([RANK, OC], FP32)
            prod3 = const.tile([RANK, OC], FP32)
            nsqrow = const.tile([1, OC], FP32)
            nrmrow = const.tile([1, OC], FP32)
            n1r = const.tile([1, OC], FP32)
            magc = const.tile([1, OC], FP32)
            srow = const.tile([1, OC], FP32)
            scrA = const.tile([128, 1024], FP32)

            for c in range(ncol_limit):
                b2tc = b2tp.tile([RANK, OC], F32R, tag="b2t", name=f"b2tc{c}")
                nc.sync.dma_start(b2tc[:], b2t_d[:, c * OC:(c + 1) * OC].bitcast(F32R))
                ps_n2 = npp.tile([RANK, OC], FP32, tag="np", name=f"psn2_{c}")

                outsb = []
                for h in range(NH):
                    wts = []
                    for j in range(H):
                        ib = h * H + j
                        w_t = wp.tile([128, OC], F32R, tag="w", name=f"w{c}_{ib}")
                        nc.sync.dma_start(w_t[:], wt_d[c, ib].bitcast(F32R))
                        wts.append(w_t)
                    # n2^T = A @ W_col^T partials [rank, OC]: A^T stationary
                    # (16-row weight load hides under the 512-row stream)
                    for j in range(H):
                        ib = h * H + j
                        nc.tensor.matmul(ps_n2[:], aT[:, ib, :], wts[j][:],
                                         start=(ib == 0), stop=(ib == NI - 1))
                    for t in range(NT):
                        ps_m = mp.tile([128, OC], FP32, tag="mp",
                                       name=f"pm{c}_{h}_{t}")
                        for j in range(H):
                            ib = h * H + j
                            nc.tensor.matmul(
                                ps_m[:], xT(ib)[:, t * 128:(t + 1) * 128], wts[j][:],
                                start=(j == 0),
                                stop=(j == H - 1 and h != NH - 1))
                        if h == NH - 1:
                            # rank-16 DoRA term folded into the accumulation
                            nc.tensor.matmul(ps_m[:],
                                             xaT[:, t * 128:(t + 1) * 128],
                                             b2tc[:], start=False, stop=True)
                        if h == 0:
                            o_t = outp.tile([128, OC], FP32, tag="o",
                                            name=f"o{c}_{t}")
                            outsb.append(o_t)
                            nc.vector.tensor_copy(o_t[:], ps_m[:])
                        else:
                            nc.vector.tensor_add(outsb[t][:], outsb[t][:], ps_m[:])

                # n1 for this column's 4 subtiles: ACT Square with row-accum
                # over fp16 W in natural layout (scalar engine is idle)
                for s in range(4):
                    osub = c * 4 + s
                    for k in range(4):
                        wh_t = whp.tile([128, 1024], FP16, tag="wh",
                                        name=f"wh{osub}_{k}")
                        nc.sync.dma_start(
                            wh_t[:], wh_d[:, osub, k * 1024:(k + 1) * 1024])
                        nc.scalar.activation(scrA[:], wh_t[:],
                                             mybir.ActivationFunctionType.Square,
                                             accum_out=n1p[:, k:k + 1])
                    nc.vector.reduce_sum(n1col[:, s:s + 1], n1p[:],
                                         axis=mybir.AxisListType.X)

                # finish norms in row space:
                #   nsq_row = n1_row + ones^T @ ((2*n2T + B2G^T) * B2T)
                ps_t = mp.tile([4, 128], FP32, tag="mp", name=f"pst{c}")
                nc.tensor.transpose(ps_t[:], n1col[:], ident[:])
                nc.vector.tensor_copy(n1row[:], ps_t[:])
                nc.sync.dma_start(n1_d[c:c + 1, :], n1row[:])
                nc.sync.dma_start(n1r[:], n1_d[c:c + 1, :])
                nc.sync.dma_start(magc[:], mag_d[:, c * OC:(c + 1) * OC])
                ps_bg = mp.tile([RANK, OC], FP32, tag="mp", name=f"psbg{c}")
                nc.tensor.matmul(ps_bg[:], g_sb[:], b2tc[:],
                                 start=True, stop=True)
                nc.vector.scalar_tensor_tensor(
                    out=prod2[:], in0=ps_n2[:], scalar=2.0,
                    in1=b2tc[:].bitcast(FP32),
                    op0=mybir.AluOpType.mult, op1=mybir.AluOpType.mult)
                nc.vector.scalar_tensor_tensor(
                    out=prod3[:], in0=ps_bg[:], scalar=1.0,
                    in1=b2tc[:].bitcast(FP32),
                    op0=mybir.AluOpType.mult, op1=mybir.AluOpType.mult)
                nc.vector.tensor_add(prod2[:], prod2[:], prod3[:])
                ps_r = mp.tile([1, OC], FP32, tag="mp", name=f"psr{c}")
                nc.tensor.matmul(ps_r[:], ones16[:], prod2[:],
                                 start=True, stop=True)
                nc.vector.tensor_add(nsqrow[:], ps_r[:], n1r[:])
                nc.scalar.activation(nrmrow[:], nsqrow[:],
                                     mybir.ActivationFunctionType.Sqrt)
                nc.vector.reciprocal(nrmrow[:], nrmrow[:])
                nc.vector.tensor_mul(srow[:], nrmrow[:], magc[:])
                sbc = sbcp.tile([128, OC], FP32, tag="sbc", name=f"sbc{c}")
                nc.sync.dma_start(srow_d[c:c + 1, :], srow[:])
                _sl = srow_d[c:c + 1, :]
                srow_bcast = bass.AP(
                    tensor=_sl.tensor, offset=_sl.offset,
                    ap=[[0, 128], [1, OC]])
                nc.gpsimd.dma_start(sbc[:], srow_bcast)

                for t in range(NT):
                    nc.vector.tensor_mul(outsb[t][:], outsb[t][:], sbc[:])
                    nc.sync.dma_start(
                        out_d[t * 128:(t + 1) * 128, c * OC:(c + 1) * OC],
                        outsb[t][:])

    nc.compile()
    return nc


_PROGRAM = None


def _get_program():
    global _PROGRAM
    if _PROGRAM is None:
        _PROGRAM = _build_program()
    return _PROGRAM


def _prep_inputs(x, weight, lora_a_w, lora_b_w, magnitude):
    xr = _round_f32r(x.reshape(TOK, DIN))
    wr = _round_f32r(weight)
    ar = _round_f32r(lora_a_w)
    b2 = _round_f32r(SCALING * lora_b_w.astype(np.float32))

    wT = np.ascontiguousarray(wr.T)                        # [in, out]
    wt = np.ascontiguousarray(
        wT.reshape(NI, 128, NCOL, OC).transpose(2, 0, 1, 3))
    wh = np.ascontiguousarray(
        wr.astype(np.float16).reshape(32, 128, DIN).transpose(1, 0, 2))
    at = np.ascontiguousarray(ar.T.reshape(NI, 128, RANK).transpose(1, 0, 2))
    b2t = np.ascontiguousarray(b2.T)
    magr = np.ascontiguousarray(
        magnitude.astype(np.float32).reshape(1, DOUT))

    xTfull = xr.T                                           # [in, tok]
    in_maps = []
    for cpu in range(NCORES):
        xs = xTfull[:, cpu * TPC:(cpu + 1) * TPC]
        xt = np.ascontiguousarray(
            xs.reshape(NI, 128, TPC).transpose(1, 0, 2))
        in_maps.append({"xt": xt, "wt": wt, "wh": wh, "at": at,
                        "b2t": b2t, "mag": magr})
    return in_maps


def kernel(x, weight, lora_a_w, lora_b_w, magnitude, _trace=False, **_kw):
    nc = _get_program()
    in_maps = _prep_inputs(x, weight, lora_a_w, lora_b_w, magnitude)
    res = run_bass_kernel_spmd(nc, in_maps, list(range(NCORES)), trace=_trace)
    out = np.concatenate([res.results[c]["out"] for c in range(NCORES)], axis=0)
    if _trace:
        kernel._last_results = res
    return out.reshape(4, 2048, DOUT)



# revision 4
# speedup vs baseline: 1.3141x; 1.3141x over previous
"""DoRA linear layer on 8 TRN2 NeuronCores.

out = (magnitude / ||W + s*B@A||_row) * (x @ (W + s*B@A)^T),  s = alpha/rank = 2.

Identity used: the reference's
    dora_out + base_out = mag_norm_scale * (base_out + s * lora_out)
                        = scale_o * (x @ W_adapted^T)
so the kernel runs ONE big fp16 matmul x @ W_ad^T (with the rank-16 term
added as an extra PSUM-accumulated matmul) and a per-out-column scale.

Sharding: data-parallel on tokens (8192 tokens -> 1024/core); W/A/B/mag
replicated. Host side only reshapes/transposes (layout prep) and casts
fp32 -> fp16 (accuracy budget is rel_err < 2e-2; fp16 gives ~3e-4).

Row norms of W_ad are computed on-device from the expansion
  ||W + B2@A||^2_row = rowsum(W*W) + 2*rowsum((W@A^T) * B2) + rowsum((B2@G) * B2)
with B2 = s*B, G = A@A^T.  rowsum(W*W) comes from ACT Square row-accumulate
over an fp16 copy of W in natural layout (scalar engine is otherwise idle);
the cross and quad terms share one PSUM accumulation (the G term is folded
in with a 0.5*G stationary), and the final reduction folds n1 in via a
17-row ones matmul.

v2 structure (vs v1): whole W column resident in SBUF as fp16 (4 MiB), so
each output tile is ONE PSUM accumulation chain of 33 matmuls followed by a
single fused scale-multiply out of PSUM; deep W prefetch removes column-
boundary stalls; norm finalization uses Rsqrt and overlaps the main chain.
"""

import sys

sys.path.insert(0, "/opt/trn_rl_repo")

import numpy as np

import concourse.bass as bass  # noqa: F401  (import keeps bass registered)
from concourse import bacc
import concourse.mybir as mybir
from concourse.tile import TileContext
from concourse.bass_utils import run_bass_kernel_spmd
from concourse.masks import make_identity

FP32 = mybir.dt.float32
F32R = mybir.dt.float32r
FP16 = mybir.dt.float16

NCORES = 8
TOK = 8192          # 4 * 2048 tokens
TPC = TOK // NCORES  # 1024 tokens per core
DIN = 4096
DOUT = 4096
RANK = 16
SCALING = 32.0 / 16

NI = DIN // 128     # 32 contraction blocks
NCOL = 8            # output columns of 512
OC = DOUT // NCOL   # 512
NT = TPC // 128     # 8 token tiles per core


def _build_program():
    nc = bacc.Bacc("TRN2", target_bir_lowering=False, debug=False,
                   num_devices=NCORES)

    xt_d = nc.dram_tensor("xt", [128, NI, TPC], FP16, kind="ExternalInput")
    wt_d = nc.dram_tensor("wt", [NCOL, NI, 128, OC], FP16, kind="ExternalInput")
    wh_d = nc.dram_tensor("wh", [128, 32, DIN], FP16, kind="ExternalInput")
    at_d = nc.dram_tensor("at", [128, NI, RANK], FP16, kind="ExternalInput")
    b2t_d = nc.dram_tensor("b2t", [RANK, DOUT], FP16, kind="ExternalInput")
    mag_d = nc.dram_tensor("mag", [1, DOUT], FP32, kind="ExternalInput")
    out_d = nc.dram_tensor("out", [TPC, DOUT], FP32, kind="ExternalOutput")
    srow_d = nc.dram_tensor("srow_scratch", [NCOL, OC], FP32)
    n1_d = nc.dram_tensor("n1_scratch", [NCOL, OC], FP32)

    with TileContext(nc) as tc:
        with (
            tc.tile_pool(name="const", bufs=1) as const,
            tc.tile_pool(name="xtp", bufs=1) as xtp,
            tc.tile_pool(name="wp", bufs=48) as wp,
            tc.tile_pool(name="outp", bufs=6) as outp,
            tc.tile_pool(name="whp", bufs=4) as whp,
            tc.tile_pool(name="sbcp", bufs=2) as sbcp,
            tc.tile_pool(name="prodp", bufs=2) as prodp,
            tc.tile_pool(name="n1cp", bufs=2) as n1cp,
            tc.tile_pool(name="mp", bufs=4, space="PSUM") as mp,
            tc.tile_pool(name="np", bufs=2, space="PSUM") as npp,
            tc.tile_pool(name="sp", bufs=2, space="PSUM") as sp,
        ):
            ident = const.tile([128, 128], FP32)
            make_identity(nc, ident)

            aT = const.tile([128, NI, RANK], FP16)
            nc.sync.dma_start(aT[:], at_d[:])
            b2t_sb = const.tile([RANK, DOUT], FP16)
            nc.sync.dma_start(b2t_sb[:], b2t_d[:])
            mag_sb = const.tile([1, DOUT], FP32)
            nc.sync.dma_start(mag_sb[:], mag_d[:])
            ones17 = const.tile([RANK + 1, 1], FP32)
            nc.vector.memset(ones17[:], 1.0)

            # resident x^T  [i_part, i_blk, tok] — four tiles so consumers
            # of early i-blocks need not wait for the whole 8 MiB load
            xTq = []
            for q in range(4):
                xq = xtp.tile([128, 8, TPC], FP16, name=f"xTq{q}")
                nc.sync.dma_start(xq[:], xt_d[:, q * 8:(q + 1) * 8, :])
                xTq.append(xq)

            def xT(ib):
                return xTq[ib // 8][:, ib % 8, :]

            # xa^T = (x @ A^T)^T  [rank, tok], cast to fp16 for the rank term
            xaT = const.tile([RANK, TPC], FP16)
            for q in range(2):
                ps_xa = sp.tile([RANK, 512], FP32, tag="sp", name=f"psxa{q}")
                for ib in range(NI):
                    nc.tensor.matmul(
                        ps_xa[:], aT[:, ib, :], xT(ib)[:, q * 512:(q + 1) * 512],
                        start=(ib == 0), stop=(ib == NI - 1))
                nc.vector.tensor_copy(xaT[:, q * 512:(q + 1) * 512], ps_xa[:])

            # g2 = 0.5 * (A @ A^T)  [rank, rank] fp16 (stationary of the
            # quad-term fold: ps_n2 += 0.5*G @ B2^T)
            ps_g = sp.tile([RANK, RANK], FP32, tag="sp", name="psg")
            for ib in range(NI):
                nc.tensor.matmul(ps_g[:], aT[:, ib, :], aT[:, ib, :],
                                 start=(ib == 0), stop=(ib == NI - 1))
            g2_sb = const.tile([RANK, RANK], FP16)
            nc.scalar.activation(g2_sb[:], ps_g[:],
                                 mybir.ActivationFunctionType.Copy, scale=0.5)

            n1p = const.tile([128, 4], FP32)
            scrA = const.tile([128, 1024], FP32)
            nrmrow = const.tile([1, OC], FP32)
            srow = const.tile([1, OC], FP32)

            for c in range(NCOL):
                b2tc = b2t_sb[:, c * OC:(c + 1) * OC]

                # --- W column load + n2 = A @ W_col^T (cross term) -------
                wts = []
                ps_n2 = npp.tile([RANK, OC], FP32, tag="np", name=f"psn2_{c}")
                for ib in range(NI):
                    w_t = wp.tile([128, OC], FP16, tag="w", name=f"w{c}_{ib}")
                    nc.sync.dma_start(w_t[:], wt_d[c, ib])
                    wts.append(w_t)
                for ib in range(NI):
                    nc.tensor.matmul(ps_n2[:], aT[:, ib, :], wts[ib][:],
                                     start=(ib == 0), stop=False)
                # quad-term fold: += (0.5*G) @ B2^T  (G symmetric)
                nc.tensor.matmul(ps_n2[:], g2_sb[:], b2tc,
                                 start=False, stop=True)

                # --- n1 = rowsum(W*W) via ACT Square row-accumulate ------
                n1col = n1cp.tile([128, 4], FP32, tag="n1c", name=f"n1c{c}")
                for s in range(4):
                    osub = c * 4 + s
                    for k in range(4):
                        wh_t = whp.tile([128, 1024], FP16, tag="wh",
                                        name=f"wh{osub}_{k}")
                        nc.sync.dma_start(
                            wh_t[:], wh_d[:, osub, k * 1024:(k + 1) * 1024])
                        nc.scalar.activation(scrA[:], wh_t[:],
                                             mybir.ActivationFunctionType.Square,
                                             accum_out=n1p[:, k:k + 1])
                    nc.vector.reduce_sum(n1col[:, s:s + 1], n1p[:],
                                         axis=mybir.AxisListType.X)
                # flatten [128,4] -> row [1,512] via transpose + DRAM bounce
                ps_t = sp.tile([4, 128], FP32, tag="sp", name=f"pst{c}")
                nc.tensor.transpose(ps_t[:], n1col[:], ident[:])
                n1row = n1cp.tile([4, 128], FP32, tag="n1r", name=f"n1r{c}")
                nc.vector.tensor_copy(n1row[:], ps_t[:])
                nc.sync.dma_start(n1_d[c:c + 1, :], n1row[:])

                # --- finish norm in row space --------------------------------
                # prod rows 0..15 = (2*ps_n2) * B2^T ; row 16 = n1 (bounced)
                prod = prodp.tile([RANK + 1, OC], FP32, tag="prod",
                                  name=f"prod{c}")
                nc.sync.dma_start(prod[RANK:RANK + 1, :], n1_d[c:c + 1, :])
                nc.vector.scalar_tensor_tensor(
                    out=prod[:RANK, :], in0=ps_n2[:], scalar=2.0,
                    in1=b2tc,
                    op0=mybir.AluOpType.mult, op1=mybir.AluOpType.mult)
                ps_r = sp.tile([1, OC], FP32, tag="sp", name=f"psr{c}")
                nc.tensor.matmul(ps_r[:], ones17[:], prod[:],
                                 start=True, stop=True)
                # scale = mag / sqrt(nsq)
                nc.scalar.activation(nrmrow[:], ps_r[:],
                                     mybir.ActivationFunctionType.Sqrt)
                nc.vector.reciprocal(nrmrow[:], nrmrow[:])
                nc.vector.tensor_mul(srow[:], nrmrow[:],
                                     mag_sb[:, c * OC:(c + 1) * OC])
                sbc = sbcp.tile([128, OC], FP32, tag="sbc", name=f"sbc{c}")
                nc.sync.dma_start(srow_d[c:c + 1, :], srow[:])
                _sl = srow_d[c:c + 1, :]
                srow_bcast = bass.AP(
                    tensor=_sl.tensor, offset=_sl.offset,
                    ap=[[0, 128], [1, OC]])
                nc.gpsimd.dma_start(sbc[:], srow_bcast)

                # --- main: one PSUM chain per token tile ---------------------
                for t in range(NT):
                    ps_m = mp.tile([128, OC], FP32, tag="mp",
                                   name=f"pm{c}_{t}")
                    for ib in range(NI):
                        nc.tensor.matmul(
                            ps_m[:], xT(ib)[:, t * 128:(t + 1) * 128],
                            wts[ib][:], start=(ib == 0), stop=False)
                    # rank-16 DoRA term folded into the accumulation
                    nc.tensor.matmul(ps_m[:],
                                     xaT[:, t * 128:(t + 1) * 128],
                                     b2tc, start=False, stop=True)
                    o_t = outp.tile([128, OC], FP32, tag="o", name=f"o{c}_{t}")
                    nc.vector.tensor_mul(o_t[:], ps_m[:], sbc[:])
                    nc.sync.dma_start(
                        out_d[t * 128:(t + 1) * 128, c * OC:(c + 1) * OC],
                        o_t[:])

    nc.compile()
    return nc


_PROGRAM = None


def _get_program():
    global _PROGRAM
    if _PROGRAM is None:
        _PROGRAM = _build_program()
    return _PROGRAM


def _prep_inputs(x, weight, lora_a_w, lora_b_w, magnitude):
    xr = np.asarray(x, dtype=np.float32).reshape(TOK, DIN)
    wr = np.asarray(weight, dtype=np.float32)
    ar = np.asarray(lora_a_w, dtype=np.float32)
    b2 = SCALING * np.asarray(lora_b_w, dtype=np.float32)

    wT = wr.T.astype(np.float16)                           # [in, out]
    wt = np.ascontiguousarray(
        wT.reshape(NI, 128, NCOL, OC).transpose(2, 0, 1, 3))
    wh = np.ascontiguousarray(
        wr.astype(np.float16).reshape(32, 128, DIN).transpose(1, 0, 2))
    at = np.ascontiguousarray(
        ar.T.astype(np.float16).reshape(NI, 128, RANK).transpose(1, 0, 2))
    b2t = np.ascontiguousarray(b2.T.astype(np.float16))
    magr = np.ascontiguousarray(
        magnitude.astype(np.float32).reshape(1, DOUT))

    xTfull = xr.T.astype(np.float16)                       # [in, tok]
    in_maps = []
    for cpu in range(NCORES):
        xs = xTfull[:, cpu * TPC:(cpu + 1) * TPC]
        xt = np.ascontiguousarray(
            xs.reshape(NI, 128, TPC).transpose(1, 0, 2))
        in_maps.append({"xt": xt, "wt": wt, "wh": wh, "at": at,
                        "b2t": b2t, "mag": magr})
    return in_maps


def kernel(x, weight, lora_a_w, lora_b_w, magnitude, _trace=False, **_kw):
    nc = _get_program()
    in_maps = _prep_inputs(x, weight, lora_a_w, lora_b_w, magnitude)
    res = run_bass_kernel_spmd(nc, in_maps, list(range(NCORES)), trace=_trace)
    out = np.concatenate([res.results[c]["out"] for c in range(NCORES)], axis=0)
    if _trace:
        kernel._last_results = res
    return out.reshape(4, 2048, DOUT)


# revision 8
# speedup vs baseline: 1.3840x; 1.0532x over previous
"""DoRA linear layer on 8 TRN2 NeuronCores.

out = (magnitude / ||W + s*B@A||_row) * (x @ (W + s*B@A)^T),  s = alpha/rank = 2.

Identity used: the reference's
    dora_out + base_out = mag_norm_scale * (base_out + s * lora_out)
                        = scale_o * (x @ W_adapted^T)
so the kernel runs ONE big fp16 matmul x @ W_ad^T (with the rank-16 term
added as an extra PSUM-accumulated matmul) and a per-out-column scale.

Sharding: data-parallel on tokens (8192 tokens -> 1024/core); W/A/B/mag
replicated. Host side only reshapes/transposes (layout prep) and casts
fp32 -> fp16 (accuracy budget is rel_err < 2e-2; fp16 gives ~3e-4).

Row norms of W_ad are computed on-device from the expansion
  ||W + B2@A||^2_row = rowsum(W*W) + 2*rowsum((W@A^T) * B2) + rowsum((B2@G) * B2)
with B2 = s*B, G = A@A^T.  rowsum(W*W) comes from ACT Square row-accumulate
over an fp16 copy of W in natural layout (scalar engine is otherwise idle);
the cross and quad terms share one PSUM accumulation (the G term is folded
in with a 0.5*G stationary), and the final reduction folds n1 in via a
17-row ones matmul.

v3 structure: whole W column resident in SBUF as fp16 with full next-column
prefetch; each output tile is ONE PSUM accumulation chain of 33 matmuls and
a single fused scale-multiply out of PSUM; the n1 (rowsum W^2) pipeline runs
one column AHEAD of the matmul column so the PE never waits on the Scalar
engine; xa = x@A^T is built from per-quarter partial chains so it paces with
the streaming x load at startup.
"""

import sys

sys.path.insert(0, "/opt/trn_rl_repo")

import numpy as np

import concourse.bass as bass  # noqa: F401  (import keeps bass registered)
from concourse import bacc
import concourse.mybir as mybir
from concourse.tile import TileContext
from concourse.bass_utils import run_bass_kernel_spmd
from concourse.masks import make_identity

FP32 = mybir.dt.float32
FP16 = mybir.dt.float16

NCORES = 8
TOK = 8192          # 4 * 2048 tokens
TPC = TOK // NCORES  # 1024 tokens per core
DIN = 4096
DOUT = 4096
RANK = 16
SCALING = 32.0 / 16

NI = DIN // 128     # 32 contraction blocks
NCOL = 8            # output columns of 512
OC = DOUT // NCOL   # 512
NT = TPC // 128     # 8 token tiles per core


def _build_program():
    nc = bacc.Bacc("TRN2", target_bir_lowering=False, debug=False,
                   num_devices=NCORES)

    xt_d = nc.dram_tensor("xt", [128, NI, TPC], FP16, kind="ExternalInput")
    wt_d = nc.dram_tensor("wt", [NCOL, NI, 128, OC], FP16, kind="ExternalInput")
    wh_d = nc.dram_tensor("wh", [128, 32, DIN], FP16, kind="ExternalInput")
    at_d = nc.dram_tensor("at", [128, NI, RANK], FP16, kind="ExternalInput")
    b2t_d = nc.dram_tensor("b2t", [RANK, DOUT], FP16, kind="ExternalInput")
    mag_d = nc.dram_tensor("mag", [1, DOUT], FP16, kind="ExternalInput")
    out_d = nc.dram_tensor("out", [TPC, DOUT], FP32, kind="ExternalOutput")
    srow_d = nc.dram_tensor("srow_scratch", [NCOL, OC], FP32)
    n1_d = nc.dram_tensor("n1_scratch", [NCOL, OC], FP16)

    with TileContext(nc) as tc:
        with (
            tc.tile_pool(name="const", bufs=1) as const,
            tc.tile_pool(name="xtp", bufs=1) as xtp,
            tc.tile_pool(name="wp", bufs=64) as wp,
            tc.tile_pool(name="outp", bufs=6) as outp,
            tc.tile_pool(name="whp", bufs=4) as whp,
            tc.tile_pool(name="sbcp", bufs=2) as sbcp,
            tc.tile_pool(name="prodp", bufs=2) as prodp,
            tc.tile_pool(name="n1cp", bufs=2) as n1cp,
            tc.tile_pool(name="xap", bufs=8) as xap,
            tc.tile_pool(name="mp", bufs=5, space="PSUM") as mp,
            tc.tile_pool(name="np", bufs=1, space="PSUM") as npp,
            tc.tile_pool(name="sp", bufs=2, space="PSUM") as sp,
        ):
            ident = const.tile([128, 128], FP32)
            make_identity(nc, ident)

            aT = const.tile([128, NI, RANK], FP16)
            nc.sync.dma_start(aT[:], at_d[:])
            b2t_sb = const.tile([RANK, DOUT], FP16)
            nc.sync.dma_start(b2t_sb[:], b2t_d[:])
            mag_sb = const.tile([1, DOUT], FP16)
            nc.sync.dma_start(mag_sb[:], mag_d[:])
            ones17 = const.tile([RANK + 1, 1], FP16)
            nc.vector.memset(ones17[:], 1.0)

            # resident x^T  [i_part, i_blk, tok] — four tiles so consumers
            # of early i-blocks need not wait for the whole 8 MiB load
            xTq = []
            for q in range(4):
                xq = xtp.tile([128, 8, TPC], FP16, name=f"xTq{q}")
                nc.sync.dma_start(xq[:], xt_d[:, q * 8:(q + 1) * 8, :])
                xTq.append(xq)

            def xT(ib):
                return xTq[ib // 8][:, ib % 8, :]

            # G first: needs only aT, so the PE has work while x streams in
            ps_g = sp.tile([RANK, RANK], FP32, tag="sp", name="psg")
            for ib in range(NI):
                nc.tensor.matmul(ps_g[:], aT[:, ib, :], aT[:, ib, :],
                                 start=(ib == 0), stop=(ib == NI - 1))
            g2_sb = const.tile([RANK, RANK], FP16)
            nc.scalar.activation(g2_sb[:], ps_g[:],
                                 mybir.ActivationFunctionType.Copy, scale=0.5)

            # xa^T = (x @ A^T)^T [rank, tok] via per-quarter partial chains
            # (each chain consumes one x quarter as it lands)
            xa_parts = []
            for q in range(4):
                for half in range(2):
                    ps_xa = sp.tile([RANK, 512], FP32, tag="sp",
                                    name=f"psxa{q}_{half}")
                    for j in range(8):
                        ib = q * 8 + j
                        nc.tensor.matmul(
                            ps_xa[:], aT[:, ib, :],
                            xT(ib)[:, half * 512:(half + 1) * 512],
                            start=(j == 0), stop=(j == 7))
                    part = xap.tile([RANK, 512], FP16, tag="xap",
                                    name=f"xap{q}_{half}")
                    nc.vector.tensor_copy(part[:], ps_xa[:])
                    xa_parts.append(part)
            xaT = const.tile([RANK, TPC], FP16)
            for half in range(2):
                hs = xaT[:, half * 512:(half + 1) * 512]
                nc.vector.tensor_add(hs, xa_parts[half][:],
                                     xa_parts[2 + half][:])
                nc.vector.tensor_add(hs, hs, xa_parts[4 + half][:])
                nc.vector.tensor_add(hs, hs, xa_parts[6 + half][:])

            n1p = const.tile([128, 4], FP32)
            scrA = const.tile([128, 1024], FP32)
            nrmrow = const.tile([1, OC], FP32)
            srow = const.tile([1, OC], FP32)

            # --- n1 pipeline helpers (run one column ahead) --------------
            def emit_n1_acc(c):
                """Scalar-engine rowsum(W*W) for column c -> n1col tile."""
                n1col = n1cp.tile([128, 4], FP32, tag="n1c", name=f"n1c{c}")
                for s in range(4):
                    osub = c * 4 + s
                    for k in range(4):
                        wh_t = whp.tile([128, 1024], FP16, tag="wh",
                                        name=f"wh{osub}_{k}")
                        nc.sync.dma_start(
                            wh_t[:], wh_d[:, osub, k * 1024:(k + 1) * 1024])
                        nc.scalar.activation(scrA[:], wh_t[:],
                                             mybir.ActivationFunctionType.Square,
                                             accum_out=n1p[:, k:k + 1])
                    nc.vector.reduce_sum(n1col[:, s:s + 1], n1p[:],
                                         axis=mybir.AxisListType.X)
                return n1col

            def emit_n1_flatten(c, n1col):
                """[128,4] -> DRAM row [1,512] via PE transpose + bounce."""
                ps_t = sp.tile([4, 128], FP32, tag="sp", name=f"pst{c}")
                nc.tensor.transpose(ps_t[:], n1col[:], ident[:])
                n1row = n1cp.tile([4, 128], FP16, tag="n1r", name=f"n1r{c}")
                nc.vector.tensor_copy(n1row[:], ps_t[:])
                nc.sync.dma_start(n1_d[c:c + 1, :], n1row[:])

            # column 0's n1 path is emitted up front (PE is busy with G/xa
            # while the Scalar engine accumulates)
            n1col0 = emit_n1_acc(0)
            emit_n1_flatten(0, n1col0)

            for c in range(NCOL):
                b2tc = b2t_sb[:, c * OC:(c + 1) * OC]

                # next column's n1 accumulation starts now on Scalar
                if c + 1 < NCOL:
                    n1col_next = emit_n1_acc(c + 1)

                # --- W column load + n2 = A @ W_col^T (cross term) -------
                wts = []
                ps_n2 = npp.tile([RANK, OC], FP32, tag="np", name=f"psn2_{c}")
                for ib in range(NI):
                    w_t = wp.tile([128, OC], FP16, tag="w", name=f"w{c}_{ib}")
                    nc.sync.dma_start(w_t[:], wt_d[c, ib])
                    wts.append(w_t)
                for ib in range(NI):
                    nc.tensor.matmul(ps_n2[:], aT[:, ib, :], wts[ib][:],
                                     start=(ib == 0), stop=False)
                # quad-term fold: += (0.5*G) @ B2^T  (G symmetric)
                nc.tensor.matmul(ps_n2[:], g2_sb[:], b2tc,
                                 start=False, stop=True)

                # --- finish norm in row space ----------------------------
                # prod rows 0..15 = (2*ps_n2) * B2^T ; row 16 = n1 (bounced
                # during the previous column)
                prod = prodp.tile([RANK + 1, OC], FP16, tag="prod",
                                  name=f"prod{c}")
                nc.sync.dma_start(prod[RANK:RANK + 1, :], n1_d[c:c + 1, :])
                nc.vector.scalar_tensor_tensor(
                    out=prod[:RANK, :], in0=ps_n2[:], scalar=2.0,
                    in1=b2tc,
                    op0=mybir.AluOpType.mult, op1=mybir.AluOpType.mult)
                ps_r = sp.tile([1, OC], FP32, tag="sp", name=f"psr{c}")
                nc.tensor.matmul(ps_r[:], ones17[:], prod[:],
                                 start=True, stop=True)
                # scale = mag / sqrt(nsq)
                nc.scalar.activation(nrmrow[:], ps_r[:],
                                     mybir.ActivationFunctionType.Sqrt)
                nc.vector.reciprocal(nrmrow[:], nrmrow[:])
                nc.vector.tensor_mul(srow[:], nrmrow[:],
                                     mag_sb[:, c * OC:(c + 1) * OC])
                sbc = sbcp.tile([128, OC], FP32, tag="sbc", name=f"sbc{c}")
                nc.sync.dma_start(srow_d[c:c + 1, :], srow[:])
                _sl = srow_d[c:c + 1, :]
                srow_bcast = bass.AP(
                    tensor=_sl.tensor, offset=_sl.offset,
                    ap=[[0, 128], [1, OC]])
                nc.gpsimd.dma_start(sbc[:], srow_bcast)

                # --- main: one PSUM chain per token tile -----------------
                for t in range(NT):
                    ps_m = mp.tile([128, OC], FP32, tag="mp",
                                   name=f"pm{c}_{t}")
                    for ib in range(NI):
                        nc.tensor.matmul(
                            ps_m[:], xT(ib)[:, t * 128:(t + 1) * 128],
                            wts[ib][:], start=(ib == 0), stop=False)
                    # rank-16 DoRA term folded into the accumulation
                    nc.tensor.matmul(ps_m[:],
                                     xaT[:, t * 128:(t + 1) * 128],
                                     b2tc, start=False, stop=True)
                    o_t = outp.tile([128, OC], FP32, tag="o", name=f"o{c}_{t}")
                    nc.vector.tensor_mul(o_t[:], ps_m[:], sbc[:])
                    nc.sync.dma_start(
                        out_d[t * 128:(t + 1) * 128, c * OC:(c + 1) * OC],
                        o_t[:])

                # flatten next column's n1 at the tail of this column's PE
                if c + 1 < NCOL:
                    emit_n1_flatten(c + 1, n1col_next)

    nc.compile()
    return nc


_PROGRAM = None


def _get_program():
    global _PROGRAM
    if _PROGRAM is None:
        _PROGRAM = _build_program()
    return _PROGRAM


def _prep_inputs(x, weight, lora_a_w, lora_b_w, magnitude):
    xr = np.asarray(x, dtype=np.float32).reshape(TOK, DIN)
    wr = np.asarray(weight, dtype=np.float32)
    ar = np.asarray(lora_a_w, dtype=np.float32)
    b2 = SCALING * np.asarray(lora_b_w, dtype=np.float32)

    wT = wr.T.astype(np.float16)                           # [in, out]
    wt = np.ascontiguousarray(
        wT.reshape(NI, 128, NCOL, OC).transpose(2, 0, 1, 3))
    wh = np.ascontiguousarray(
        wr.astype(np.float16).reshape(32, 128, DIN).transpose(1, 0, 2))
    at = np.ascontiguousarray(
        ar.T.astype(np.float16).reshape(NI, 128, RANK).transpose(1, 0, 2))
    b2t = np.ascontiguousarray(b2.T.astype(np.float16))
    magr = np.ascontiguousarray(
        magnitude.astype(np.float16).reshape(1, DOUT))

    xTfull = xr.T.astype(np.float16)                       # [in, tok]
    in_maps = []
    for cpu in range(NCORES):
        xs = xTfull[:, cpu * TPC:(cpu + 1) * TPC]
        xt = np.ascontiguousarray(
            xs.reshape(NI, 128, TPC).transpose(1, 0, 2))
        in_maps.append({"xt": xt, "wt": wt, "wh": wh, "at": at,
                        "b2t": b2t, "mag": magr})
    return in_maps


def kernel(x, weight, lora_a_w, lora_b_w, magnitude, _trace=False, **_kw):
    nc = _get_program()
    in_maps = _prep_inputs(x, weight, lora_a_w, lora_b_w, magnitude)
    res = run_bass_kernel_spmd(nc, in_maps, list(range(NCORES)), trace=_trace)
    out = np.concatenate([res.results[c]["out"] for c in range(NCORES)], axis=0)
    if _trace:
        kernel._last_results = res
    return out.reshape(4, 2048, DOUT)


# revision 11
# speedup vs baseline: 1.5492x; 1.1194x over previous
"""DoRA linear layer on 8 TRN2 NeuronCores.

out = (magnitude / ||W + s*B@A||_row) * (x @ (W + s*B@A)^T),  s = alpha/rank = 2.

Identity used: the reference's
    dora_out + base_out = mag_norm_scale * (base_out + s * lora_out)
                        = scale_o * (x @ W_adapted^T)
so the kernel runs ONE big fp16 matmul x @ W_ad^T (with the rank-16 term
added as an extra PSUM-accumulated matmul) and a per-out-column scale.

Sharding: data-parallel on tokens for the GEMM (8192 tokens -> 1024/core,
W/A/B replicated), BUT the row-norm computation is sharded on out_dim:
core k computes the scale vector for output column k only (from small
per-core staged inputs wtn/whn/b2n/magn), and one 16 KiB AllGather
distributes all 8 scale rows to every core.  This removes 7/8 of the
norm matmul work and all per-column norm serialization from the main loop.

Row norm of W_ad for my column, on-device:
  ||W + B2@A||^2_row = rowsum(W*W) + 2*rowsum((W@A^T) * B2) + rowsum((B2@G) * B2)
with B2 = s*B, G = A@A^T.  rowsum(W*W) comes from ACT Square row-accumulate
over an fp16 copy of my W rows in natural layout; the cross and quad terms
share one PSUM accumulation (G folded in with a 0.5*G stationary); the final
reduction folds n1 in via a 17-row ones matmul.

Main loop: per column, 8 token tiles, each ONE PSUM chain of 33 fp16
matmuls; PSUM is drained by a plain copy; the scale multiply runs one
column behind (when the AllGather result is guaranteed ready) and feeds
the output DMA.
"""

import sys

sys.path.insert(0, "/opt/trn_rl_repo")

import numpy as np

import concourse.bass as bass  # noqa: F401  (import keeps bass registered)
from concourse import bacc
import concourse.mybir as mybir
from concourse.tile import TileContext
from concourse.bass_utils import run_bass_kernel_spmd
from concourse.masks import make_identity

FP32 = mybir.dt.float32
FP16 = mybir.dt.float16

NCORES = 8
TOK = 8192          # 4 * 2048 tokens
TPC = TOK // NCORES  # 1024 tokens per core
DIN = 4096
DOUT = 4096
RANK = 16
SCALING = 32.0 / 16

NI = DIN // 128     # 32 contraction blocks
NCOL = 8            # output columns of 512
OC = DOUT // NCOL   # 512
NT = TPC // 128     # 8 token tiles per core


def _build_program():
    nc = bacc.Bacc("TRN2", target_bir_lowering=False, debug=False,
                   num_devices=NCORES)

    xt_d = nc.dram_tensor("xt", [128, NI, TPC], FP16, kind="ExternalInput")
    wt_d = nc.dram_tensor("wt", [NCOL, NI, 128, OC], FP16, kind="ExternalInput")
    wtn_d = nc.dram_tensor("wtn", [NI, 128, OC], FP16, kind="ExternalInput")
    whn_d = nc.dram_tensor("whn", [128, 4, DIN], FP16, kind="ExternalInput")
    at_d = nc.dram_tensor("at", [128, NI, RANK], FP16, kind="ExternalInput")
    b2t_d = nc.dram_tensor("b2t", [RANK, DOUT], FP16, kind="ExternalInput")
    b2n_d = nc.dram_tensor("b2n", [RANK, OC], FP16, kind="ExternalInput")
    magn_d = nc.dram_tensor("magn", [1, OC], FP16, kind="ExternalInput")
    out_d = nc.dram_tensor("out", [TPC, DOUT], FP32, kind="ExternalOutput")
    n1_d = nc.dram_tensor("n1_scratch", [1, OC], FP16)

    with TileContext(nc) as tc:
        with (
            tc.tile_pool(name="const", bufs=1) as const,
            tc.tile_pool(name="xtp", bufs=1) as xtp,
            tc.tile_pool(name="wp", bufs=64) as wp,
            tc.tile_pool(name="wtnp", bufs=8) as wtnp,
            tc.tile_pool(name="outp", bufs=12) as outp,
            tc.tile_pool(name="whp", bufs=4) as whp,
            tc.tile_pool(name="sbcp", bufs=8) as sbcp,
            tc.tile_pool(name="xap", bufs=8) as xap,
            tc.tile_pool(name="dram", bufs=1, space="DRAM") as dram,
            tc.tile_pool(name="mp", bufs=5, space="PSUM") as mp,
            tc.tile_pool(name="np", bufs=1, space="PSUM") as npp,
            tc.tile_pool(name="sp", bufs=2, space="PSUM") as sp,
        ):
            ident = const.tile([128, 128], FP32)
            make_identity(nc, ident)

            aT = const.tile([128, NI, RANK], FP16)
            nc.sync.dma_start(aT[:], at_d[:])
            b2t_sb = const.tile([RANK, DOUT], FP16)
            nc.sync.dma_start(b2t_sb[:], b2t_d[:])
            b2n_sb = const.tile([RANK, OC], FP16)
            nc.sync.dma_start(b2n_sb[:], b2n_d[:])
            magn_sb = const.tile([1, OC], FP16)
            nc.sync.dma_start(magn_sb[:], magn_d[:])
            ones17 = const.tile([RANK + 1, 1], FP16)
            nc.vector.memset(ones17[:], 1.0)

            # resident x^T  [i_part, i_blk, tok] — four tiles so consumers
            # of early i-blocks need not wait for the whole 8 MiB load
            xTq = []
            for q in range(4):
                xq = xtp.tile([128, 8, TPC], FP16, name=f"xTq{q}")
                nc.sync.dma_start(xq[:], xt_d[:, q * 8:(q + 1) * 8, :])
                xTq.append(xq)

            def xT(ib):
                return xTq[ib // 8][:, ib % 8, :]

            # G first: needs only aT, so the PE has work immediately
            ps_g = sp.tile([RANK, RANK], FP32, tag="sp", name="psg")
            for ib in range(NI):
                nc.tensor.matmul(ps_g[:], aT[:, ib, :], aT[:, ib, :],
                                 start=(ib == 0), stop=(ib == NI - 1))
            g2_sb = const.tile([RANK, RANK], FP16)
            nc.scalar.activation(g2_sb[:], ps_g[:],
                                 mybir.ActivationFunctionType.Copy, scale=0.5)

            # --- my-column norm: n2 = A @ W_mycol^T (+ 0.5*G @ B2^T) -----
            ps_n2 = npp.tile([RANK, OC], FP32, tag="n2", name="psn2")
            for ib in range(NI):
                wn_t = wtnp.tile([128, OC], FP16, tag="wtn", name=f"wtn{ib}")
                nc.sync.dma_start(wn_t[:], wtn_d[ib])
                nc.tensor.matmul(ps_n2[:], aT[:, ib, :], wn_t[:],
                                 start=(ib == 0), stop=False)
            nc.tensor.matmul(ps_n2[:], g2_sb[:], b2n_sb[:],
                             start=False, stop=True)

            # --- my-column n1 = rowsum(W*W) via ACT Square ---------------
            n1p = const.tile([128, 4], FP32)
            scrA = const.tile([128, 1024], FP32)
            n1col = const.tile([128, 4], FP32)
            for s in range(4):
                for k in range(4):
                    wh_t = whp.tile([128, 1024], FP16, tag="wh",
                                    name=f"wh{s}_{k}")
                    nc.sync.dma_start(
                        wh_t[:], whn_d[:, s, k * 1024:(k + 1) * 1024])
                    nc.scalar.activation(scrA[:], wh_t[:],
                                         mybir.ActivationFunctionType.Square,
                                         accum_out=n1p[:, k:k + 1])
                nc.vector.reduce_sum(n1col[:, s:s + 1], n1p[:],
                                     axis=mybir.AxisListType.X)
            # flatten [128,4] -> row [1,512] via transpose + DRAM bounce
            ps_t = sp.tile([4, 128], FP32, tag="sp", name="pst")
            nc.tensor.transpose(ps_t[:], n1col[:], ident[:])
            n1row = const.tile([4, 128], FP16)
            nc.vector.tensor_copy(n1row[:], ps_t[:])
            nc.sync.dma_start(n1_d[:], n1row[:])

            # --- finish my scale in row space ----------------------------
            prod = const.tile([RANK + 1, OC], FP16)
            nc.sync.dma_start(prod[RANK:RANK + 1, :], n1_d[:])
            nc.vector.scalar_tensor_tensor(
                out=prod[:RANK, :], in0=ps_n2[:], scalar=2.0,
                in1=b2n_sb[:],
                op0=mybir.AluOpType.mult, op1=mybir.AluOpType.mult)
            ps_r = sp.tile([1, OC], FP32, tag="sp", name="psr")
            nc.tensor.matmul(ps_r[:], ones17[:], prod[:],
                             start=True, stop=True)
            nrmrow = const.tile([1, OC], FP32)
            srow = const.tile([1, OC], FP32)
            nc.scalar.activation(nrmrow[:], ps_r[:],
                                 mybir.ActivationFunctionType.Sqrt)
            nc.vector.reciprocal(nrmrow[:], nrmrow[:])
            nc.vector.tensor_mul(srow[:], nrmrow[:], magn_sb[:])

            # --- AllGather: scale rows of all 8 columns ------------------
            srow_my = dram.tile([1, OC], FP32, name="srow_my")
            srow_all = dram.tile([NCORES, OC], FP32, name="srow_all")
            nc.gpsimd.dma_start(srow_my[:], srow[:])
            nc.gpsimd.collective_compute(
                "AllGather",
                mybir.AluOpType.bypass,
                replica_groups=[list(range(NCORES))],
                ins=[srow_my[:].opt()],
                outs=[srow_all[:].opt()],
            )
            sbcs = []
            for c in range(NCOL):
                sbc = sbcp.tile([128, OC], FP16, tag="sbc", name=f"sbc{c}")
                _sl = srow_all[c:c + 1, :]
                srow_bcast = bass.AP(
                    tensor=_sl.tensor, offset=_sl.offset,
                    ap=[[0, 128], [1, OC]])
                nc.gpsimd.dma_start(sbc[:], srow_bcast)
                sbcs.append(sbc)

            # xa^T = (x @ A^T)^T [rank, tok] via per-quarter partial chains
            xa_parts = []
            for q in range(4):
                for half in range(2):
                    ps_xa = sp.tile([RANK, 512], FP32, tag="sp",
                                    name=f"psxa{q}_{half}")
                    for j in range(8):
                        ib = q * 8 + j
                        nc.tensor.matmul(
                            ps_xa[:], aT[:, ib, :],
                            xT(ib)[:, half * 512:(half + 1) * 512],
                            start=(j == 0), stop=(j == 7))
                    part = xap.tile([RANK, 512], FP16, tag="xap",
                                    name=f"xap{q}_{half}")
                    nc.vector.tensor_copy(part[:], ps_xa[:])
                    xa_parts.append(part)
            xaT = const.tile([RANK, TPC], FP16)
            for half in range(2):
                hs = xaT[:, half * 512:(half + 1) * 512]
                nc.vector.tensor_add(hs, xa_parts[half][:],
                                     xa_parts[2 + half][:])
                nc.vector.tensor_add(hs, hs, xa_parts[4 + half][:])
                nc.vector.tensor_add(hs, hs, xa_parts[6 + half][:])

            # --- main loop: pure matmul chains; scale lags one column ----
            held = []  # (c, t, o_t) awaiting scale multiply

            for c in range(NCOL):
                b2tc = b2t_sb[:, c * OC:(c + 1) * OC]
                wts = []
                for ib in range(NI):
                    w_t = wp.tile([128, OC], FP16, tag="w", name=f"w{c}_{ib}")
                    nc.sync.dma_start(w_t[:], wt_d[c, ib])
                    wts.append(w_t)
                prev_held = held[:]
                held = []
                for t in range(NT):
                    ps_m = mp.tile([128, OC], FP32, tag="mp",
                                   name=f"pm{c}_{t}")
                    for ib in range(NI):
                        nc.tensor.matmul(
                            ps_m[:], xT(ib)[:, t * 128:(t + 1) * 128],
                            wts[ib][:], start=(ib == 0), stop=False)
                    # rank-16 DoRA term folded into the accumulation
                    nc.tensor.matmul(ps_m[:],
                                     xaT[:, t * 128:(t + 1) * 128],
                                     b2tc, start=False, stop=True)
                    o_t = outp.tile([128, OC], FP32, tag="o", name=f"o{c}_{t}")
                    nc.vector.tensor_copy(o_t[:], ps_m[:])
                    held.append((c, t, o_t))
                    # previous column's scale multiply, interleaved
                    if prev_held:
                        hc, ht, ho = prev_held.pop(0)
                        nc.vector.tensor_mul(ho[:], ho[:], sbcs[hc][:])
                        nc.sync.dma_start(
                            out_d[ht * 128:(ht + 1) * 128,
                                  hc * OC:(hc + 1) * OC],
                            ho[:])
            for (hc, ht, ho) in held:
                nc.vector.tensor_mul(ho[:], ho[:], sbcs[hc][:])
                nc.sync.dma_start(
                    out_d[ht * 128:(ht + 1) * 128, hc * OC:(hc + 1) * OC],
                    ho[:])

    nc.compile()
    return nc


_PROGRAM = None


def _get_program():
    global _PROGRAM
    if _PROGRAM is None:
        _PROGRAM = _build_program()
    return _PROGRAM


def _prep_inputs(x, weight, lora_a_w, lora_b_w, magnitude):
    xr = np.asarray(x, dtype=np.float32).reshape(TOK, DIN)
    wr = np.asarray(weight, dtype=np.float32)
    ar = np.asarray(lora_a_w, dtype=np.float32)
    b2 = SCALING * np.asarray(lora_b_w, dtype=np.float32)

    wT = wr.T.astype(np.float16)                           # [in, out]
    wt = np.ascontiguousarray(
        wT.reshape(NI, 128, NCOL, OC).transpose(2, 0, 1, 3))
    wh16 = wr.astype(np.float16)
    at = np.ascontiguousarray(
        ar.T.astype(np.float16).reshape(NI, 128, RANK).transpose(1, 0, 2))
    b2t = np.ascontiguousarray(b2.T.astype(np.float16))
    mag16 = magnitude.astype(np.float16).reshape(1, DOUT)

    xTfull = xr.T.astype(np.float16)                       # [in, tok]
    in_maps = []
    for cpu in range(NCORES):
        xs = xTfull[:, cpu * TPC:(cpu + 1) * TPC]
        xt = np.ascontiguousarray(
            xs.reshape(NI, 128, TPC).transpose(1, 0, 2))
        whn = np.ascontiguousarray(
            wh16[cpu * OC:(cpu + 1) * OC]
            .reshape(4, 128, DIN).transpose(1, 0, 2))
        in_maps.append({
            "xt": xt, "wt": wt, "at": at, "b2t": b2t,
            "wtn": np.ascontiguousarray(wt[cpu]),
            "whn": whn,
            "b2n": np.ascontiguousarray(b2t[:, cpu * OC:(cpu + 1) * OC]),
            "magn": np.ascontiguousarray(mag16[:, cpu * OC:(cpu + 1) * OC]),
        })
    return in_maps


def kernel(x, weight, lora_a_w, lora_b_w, magnitude, _trace=False, **_kw):
    nc = _get_program()
    in_maps = _prep_inputs(x, weight, lora_a_w, lora_b_w, magnitude)
    res = run_bass_kernel_spmd(nc, in_maps, list(range(NCORES)), trace=_trace)
    out = np.concatenate([res.results[c]["out"] for c in range(NCORES)], axis=0)
    if _trace:
        kernel._last_results = res
    return out.reshape(4, 2048, DOUT)


# revision 12
# speedup vs baseline: 1.7601x; 1.1362x over previous
"""DoRA linear layer on 8 TRN2 NeuronCores.

out = (magnitude / ||W + s*B@A||_row) * (x @ (W + s*B@A)^T),  s = alpha/rank = 2.

Identity used: the reference's
    dora_out + base_out = mag_norm_scale * (base_out + s * lora_out)
                        = scale_o * (x @ W_adapted^T)

Sharding: TENSOR-PARALLEL on out_dim (per the sharding hint): core k owns
output columns [512k, 512(k+1)), x is replicated (streamed), W/lora_b/
magnitude are column-sharded.  This makes the norm/scale computation fully
LOCAL to each core — no cross-core exchange of any kind.

On-device, each core materializes its adapted weight column ONCE:
    W_ad^T = W^T + A^T @ B2^T      (32 K=16 matmuls + 32 DVE adds, fp16)
after which
  * the main GEMM is 64 token-tiles x ONE PSUM chain of 32 fp16 matmuls
    (no separate rank-16 path, no x@A^T precompute), and
  * the row norm is simply rowsum(W_ad^2): 32 DVE squares + 32 ones-matmul
    accumulations into a [1,512] PSUM, consistent to the bit with the
    weights the GEMM consumes.
scale = mag / sqrt(nsq) broadcast once into a [128,512] tile; every PSUM
drain is a single fused tensor_mul.

Host side only reshapes/transposes (layout prep), casts fp32 -> fp16
(accuracy budget is rel_err < 2e-2; fp16 gives ~4e-4), and concatenates
the per-core output column blocks.
"""

import sys

sys.path.insert(0, "/opt/trn_rl_repo")

import numpy as np

import concourse.bass as bass  # noqa: F401  (import keeps bass registered)
from concourse import bacc
import concourse.mybir as mybir
from concourse.tile import TileContext
from concourse.bass_utils import run_bass_kernel_spmd

FP32 = mybir.dt.float32
FP16 = mybir.dt.float16

NCORES = 8
TOK = 8192          # 4 * 2048 tokens total, all processed by every core
DIN = 4096
DOUT = 4096
RANK = 16
SCALING = 32.0 / 16

NI = DIN // 128     # 32 contraction blocks
OC = DOUT // NCORES  # 512 output columns per core
NB = TOK // 128     # 64 token tiles per core


def _build_program():
    nc = bacc.Bacc("TRN2", target_bir_lowering=False, debug=False,
                   num_devices=NCORES)

    # x in token-block-major layout: block t -> [128 part, NI*128] contiguous
    xb_d = nc.dram_tensor("xb", [NB, 128, NI * 128], FP16,
                          kind="ExternalInput")
    wt_d = nc.dram_tensor("wt", [NI, 128, OC], FP16, kind="ExternalInput")
    atr_d = nc.dram_tensor("atr", [RANK, NI, 128], FP16, kind="ExternalInput")
    b2n_d = nc.dram_tensor("b2n", [RANK, OC], FP16, kind="ExternalInput")
    magn_d = nc.dram_tensor("magn", [1, OC], FP32, kind="ExternalInput")
    out_d = nc.dram_tensor("out", [TOK, OC], FP32, kind="ExternalOutput")
    srow_d = nc.dram_tensor("srow_scratch", [1, OC], FP32)

    with TileContext(nc) as tc:
        with (
            tc.tile_pool(name="const", bufs=1) as const,
            tc.tile_pool(name="xbp", bufs=8) as xbp,
            tc.tile_pool(name="wp", bufs=6) as wp,
            tc.tile_pool(name="wadp", bufs=32) as wadp,
            tc.tile_pool(name="wsqp", bufs=4) as wsqp,
            tc.tile_pool(name="outp", bufs=10) as outp,
            tc.tile_pool(name="mp", bufs=6, space="PSUM") as mp,
            tc.tile_pool(name="sp", bufs=2, space="PSUM") as sp,
        ):
            atr = const.tile([RANK, NI, 128], FP16)
            nc.sync.dma_start(atr[:], atr_d[:])
            b2n_sb = const.tile([RANK, OC], FP16)
            nc.sync.dma_start(b2n_sb[:], b2n_d[:])
            magn_sb = const.tile([1, OC], FP32)
            nc.sync.dma_start(magn_sb[:], magn_d[:])
            ones128 = const.tile([128, 1], FP16)
            nc.vector.memset(ones128[:], 1.0)

            # --- W_ad^T = W^T + A^T @ B2^T, per 128-row i-block ----------
            # also accumulate nsq = colsum(W_ad^2) as the blocks appear
            wads = []
            ps_nsq = sp.tile([1, OC], FP32, tag="nsq", name="psnsq")
            for ib in range(NI):
                w_t = wp.tile([128, OC], FP16, tag="w", name=f"w{ib}")
                nc.sync.dma_start(w_t[:], wt_d[ib])
                ps_l = mp.tile([128, OC], FP32, tag="mp", name=f"pl{ib}")
                nc.tensor.matmul(ps_l[:], atr[:, ib, :], b2n_sb[:],
                                 start=True, stop=True)
                wad = wadp.tile([128, OC], FP16, tag="wad", name=f"wad{ib}")
                nc.vector.tensor_add(wad[:], ps_l[:], w_t[:])
                wads.append(wad)
                wsq = wsqp.tile([128, OC], FP16, tag="wsq", name=f"wsq{ib}")
                nc.vector.tensor_mul(wsq[:], wad[:], wad[:])
                nc.tensor.matmul(ps_nsq[:], ones128[:], wsq[:],
                                 start=(ib == 0), stop=(ib == NI - 1))

            # --- scale = mag / sqrt(nsq), broadcast over partitions ------
            nrmrow = const.tile([1, OC], FP32)
            srow = const.tile([1, OC], FP32)
            nc.scalar.activation(nrmrow[:], ps_nsq[:],
                                 mybir.ActivationFunctionType.Sqrt)
            nc.vector.reciprocal(nrmrow[:], nrmrow[:])
            nc.vector.tensor_mul(srow[:], nrmrow[:], magn_sb[:])
            sbc = const.tile([128, OC], FP32)
            nc.gpsimd.dma_start(srow_d[:], srow[:])
            _sl = srow_d[:]
            srow_bcast = bass.AP(
                tensor=_sl.tensor, offset=_sl.offset,
                ap=[[0, 128], [1, OC]])
            nc.gpsimd.dma_start(sbc[:], srow_bcast)

            # --- main GEMM: 64 token tiles, one PSUM chain each ----------
            for t in range(NB):
                xb = xbp.tile([128, NI * 128], FP16, tag="xb", name=f"xb{t}")
                nc.sync.dma_start(xb[:], xb_d[t])
                ps_m = mp.tile([128, OC], FP32, tag="mp", name=f"pm{t}")
                for ib in range(NI):
                    nc.tensor.matmul(
                        ps_m[:], xb[:, ib * 128:(ib + 1) * 128],
                        wads[ib][:], start=(ib == 0), stop=(ib == NI - 1))
                o_t = outp.tile([128, OC], FP32, tag="o", name=f"o{t}")
                nc.vector.tensor_mul(o_t[:], ps_m[:], sbc[:])
                nc.sync.dma_start(
                    out_d[t * 128:(t + 1) * 128, :], o_t[:])

    nc.compile()
    return nc


_PROGRAM = None


def _get_program():
    global _PROGRAM
    if _PROGRAM is None:
        _PROGRAM = _build_program()
    return _PROGRAM


def _prep_inputs(x, weight, lora_a_w, lora_b_w, magnitude):
    xr = np.asarray(x, dtype=np.float32).reshape(TOK, DIN)
    wr = np.asarray(weight, dtype=np.float32)
    ar = np.asarray(lora_a_w, dtype=np.float32)
    b2 = SCALING * np.asarray(lora_b_w, dtype=np.float32)

    # x token-block-major: [NB, 128 part(i%128), NI*128] per token block
    xT = xr.T.astype(np.float16)                       # [in, tok]
    xb = np.ascontiguousarray(
        xT.reshape(NI, 128, NB, 128).transpose(2, 1, 0, 3)
        .reshape(NB, 128, NI * 128))

    wT = wr.T.astype(np.float16)                       # [in, out]
    atr = np.ascontiguousarray(ar.astype(np.float16).reshape(RANK, NI, 128))
    b2t = b2.T.astype(np.float16)                      # [rank, out]
    mag32 = magnitude.astype(np.float32).reshape(1, DOUT)

    in_maps = []
    for cpu in range(NCORES):
        cs = slice(cpu * OC, (cpu + 1) * OC)
        wt = np.ascontiguousarray(wT[:, cs].reshape(NI, 128, OC))
        in_maps.append({
            "xb": xb, "wt": wt, "atr": atr,
            "b2n": np.ascontiguousarray(b2t[:, cs]),
            "magn": np.ascontiguousarray(mag32[:, cs]),
        })
    return in_maps


def kernel(x, weight, lora_a_w, lora_b_w, magnitude, _trace=False, **_kw):
    nc = _get_program()
    in_maps = _prep_inputs(x, weight, lora_a_w, lora_b_w, magnitude)
    res = run_bass_kernel_spmd(nc, in_maps, list(range(NCORES)), trace=_trace)
    out = np.concatenate([res.results[c]["out"] for c in range(NCORES)],
                         axis=1)
    if _trace:
        kernel._last_results = res
    return out.reshape(4, 2048, DOUT)
